# revision 1
# baseline (speedup 1.0000x reference)
"""Trainium2 Bass kernel for nn_Attention_6468220748045.

Computes, per batch item: QKV projection -> per-head scaled attention with a
multiplicative positional bias w[i,j] = |i-j|/S -> softmax -> attn @ V ->
LayerNorm over the embedding dim.

Sharding: pure data-parallel over batch. B=128 splits as 16 batch items per
core across 8 NeuronCores; no collectives needed. Inputs are pre-laid-out on
host: x is passed transposed per batch ([B, E, S]) so both projection
orientations stream directly from SBUF, and the weights are passed transposed
([e_in, e_out]) to serve as matmul stationary operands.

Per-core kernel layout choices:
  - QT/KT projections: stationary = W.T tile [e_in,128 x e_out,128], moving =
    x.T for a PAIR of batches ([e_in,128 x 358]) -> fp32r runs at full rate
    (moving dim >= 256). Output orientation [e_out, s] is exactly what the
    scores matmul needs (contraction over head dim on the partition axis).
  - V projection: stationary = x.T tile, moving = Wv.T ([e_in,128 x 512]),
    giving V in natural [s, e] orientation for the PV matmul.
  - Scores: s.T[j, i] = (k_h).T.T @ (q_h).T in bf16, multiplied by the
    host-precomputed scale*w[j, i], exponentiated on ScalarE (no max
    subtraction: |scores| <= ~2, exp is safe).
  - Softmax denominator comes for free from the PV matmul: V is stored padded
    [s, H, 65] with a ones column, so out[:, 64] = sum_j p[j, i].
  - PV: stationary = p.T tile (bf16), moving = padded V tile; normalize by the
    reciprocal of the ones-column and write straight into the [s, e] output
    tile, which then gets LayerNorm'd (bn_stats/bn_aggr) and DMA'd out.
"""

import numpy as np

import concourse.bass as bass
import concourse.tile as tile
from concourse import bacc, mybir
from concourse.bass_utils import run_bass_kernel_spmd

# Problem constants (hardcoded per the self-contained-kernel contract).
B, S, E, H, D = 128, 179, 1024, 16, 64
NCORES = 8
BPC = B // NCORES          # batches per core = 16
NPAIR = BPC // 2           # batch pairs per core = 8
KT = E // 128              # contraction tiles over e_in = 8
MT = E // 128              # output tiles over e_out = 8
S0 = 128                   # first s-tile size
S1 = S - S0                # second s-tile size = 51
S_TILES = ((0, S0), (S0, S1))
LN_EPS = 1e-5
SCALE = float(E) ** -0.5

F32 = mybir.dt.float32
F32R = mybir.dt.float32r
BF16 = mybir.dt.bfloat16

AF = mybir.ActivationFunctionType
ALU = mybir.AluOpType


def _build_kernel(bpc: int = BPC, apply_gb: bool = True) -> bass.Bass:
    npair = bpc // 2
    nc = bacc.Bacc()

    xT = nc.dram_tensor("xT", [bpc, E, S], BF16, kind="ExternalInput").ap()
    wqT = nc.dram_tensor("wqT", [E, E], BF16, kind="ExternalInput").ap()
    wkT = nc.dram_tensor("wkT", [E, E], BF16, kind="ExternalInput").ap()
    wvT = nc.dram_tensor("wvT", [E, E], BF16, kind="ExternalInput").ap()
    wsc = nc.dram_tensor("wsc", [S, S], F32, kind="ExternalInput").ap()
    gamma = nc.dram_tensor("gamma", [E], F32, kind="ExternalInput").ap()
    beta = nc.dram_tensor("beta", [E], F32, kind="ExternalInput").ap()
    out = nc.dram_tensor("out", [bpc, S, E], F32, kind="ExternalOutput").ap()

    with tile.TileContext(nc) as tc:
        _emit(tc, npair, out, xT, wqT, wkT, wvT, wsc, gamma, beta, apply_gb)
    nc.compile()
    return nc


def _emit(tc, npair, out, xT, wqT, wkT, wvT, wsc, gamma, beta, apply_gb):
    nc = tc.nc
    from contextlib import ExitStack

    with ExitStack() as ctx:
        singles = ctx.enter_context(tc.tile_pool(name="singles", bufs=1))
        xt_pool = ctx.enter_context(tc.tile_pool(name="xt", bufs=3))
        qk_pool = ctx.enter_context(tc.tile_pool(name="qk", bufs=3))
        v_pool = ctx.enter_context(tc.tile_pool(name="v", bufs=6))
        p_pool = ctx.enter_context(tc.tile_pool(name="p", bufs=4))
        o_pool = ctx.enter_context(tc.tile_pool(name="o", bufs=3))
        ln_pool = ctx.enter_context(tc.tile_pool(name="ln", bufs=4))
        r_pool = ctx.enter_context(tc.tile_pool(name="r", bufs=8))

        # QK and V projection psums share one pool/tag (they alternate within
        # proj_gen), freeing a PSUM bank so the scores pool gets 3 buffers —
        # scores matmuls can then run further ahead of the w-mul/exp chain.
        pp_proj = ctx.enter_context(tc.tile_pool(name="pp_proj", bufs=3, space="PSUM"))
        pp_qk = pp_v = pp_proj
        pp_s = ctx.enter_context(tc.tile_pool(name="pp_s", bufs=3, space="PSUM"))
        pp_o = ctx.enter_context(tc.tile_pool(name="pp_o", bufs=2, space="PSUM"))

        # --- resident tensors -------------------------------------------------
        # Weight tiles: [e_in partition, k-tile, e_out]. DMA order matters for
        # startup latency: wq first, then pair-0's x.T, then wk/wv — the first
        # Q.T matmuls only need wq + x.T.
        xsrc = xT.rearrange("b (k p) s -> k p b s", p=128)  # [KT, 128, bpc, S]
        w_sbs = []
        for name, wap in (("wq", wqT), ("wk", wkT), ("wv", wvT)):
            w_sb = singles.tile([128, KT, E], BF16, tag=f"w_{name}")
            w_sbs.append(w_sb)
        wq_sb, wk_sb, wv_sb = w_sbs
        xt0 = xt_pool.tile([128, KT, 2, S], BF16, tag="xt", name="xt_0")
        for w_sb, wap in ((wq_sb, wqT),):
            src = wap.rearrange("(k p) e -> k p e", p=128)
            for k in range(KT):
                nc.sync.dma_start(out=w_sb[:, k], in_=src[k])
        for k in range(KT):
            nc.sync.dma_start(out=xt0[:, k], in_=xsrc[k, :, 0:2, :])
        for w_sb, wap in ((wk_sb, wkT), (wv_sb, wvT)):
            src = wap.rearrange("(k p) e -> k p e", p=128)
            for k in range(KT):
                nc.sync.dma_start(out=w_sb[:, k], in_=src[k])

        # Positional bias (already includes softmax scale): [j partition, jt, i]
        wsc_sb = singles.tile([128, 2, S], F32, tag="wsc")
        nc.vector.memset(wsc_sb[:, 1], 0.0)
        nc.sync.dma_start(out=wsc_sb[:, 0], in_=wsc[0:S0])
        nc.sync.dma_start(out=wsc_sb[0:S1, 1], in_=wsc[S0:S])

        # gamma/beta broadcast to all partitions (skipped when the caller
        # verified they are identity); eps scalar.
        if apply_gb:
            gamma_b = singles.tile([128, E], F32, tag="gamma")
            beta_b = singles.tile([128, E], F32, tag="beta")
            nc.sync.dma_start(
                out=gamma_b,
                in_=bass.AP(tensor=gamma.tensor, offset=gamma.offset, ap=[[0, 128]] + gamma.ap),
            )
            nc.sync.dma_start(
                out=beta_b,
                in_=bass.AP(tensor=beta.tensor, offset=beta.offset, ap=[[0, 128]] + beta.ap),
            )
        eps_t = singles.tile([128, 1], F32, tag="eps")
        nc.vector.memset(eps_t, LN_EPS)

        # Per-pair SBUF products handed from the projection stage to the
        # attention stage (software pipeline).
        stage: dict = {}

        def proj_gen(pr):
            """QKV projections for batch pair `pr`; yields after each PE chunk
            (~8 matmuls) so attention of pair pr-1 can be interleaved."""
            if pr == 0:
                xt = xt0
            else:
                xt = xt_pool.tile([128, KT, 2, S], BF16, tag="xt", name=f"xt_{pr}")
                for k in range(KT):
                    nc.sync.dma_start(
                        out=xt[:, k], in_=xsrc[k, :, 2 * pr : 2 * pr + 2, :]
                    )

            # Q.T / K.T: out[e_out, s2], s2 = 2*S = 358 (both batches at once).
            # kt is stored s-padded to 2*128 per batch with a zeroed tail, so
            # the jt=1 scores matmul has a full 128-wide stationary and writes
            # all 128 psum rows (zeros beyond row 51) — no per-head memset.
            qt_sb = qk_pool.tile([128, MT, 2, S], BF16, tag="qt", name=f"qt_{pr}")
            kt_sb = qk_pool.tile([128, MT, 2, 2 * 128], BF16, tag="kt", name=f"kt_{pr}")
            nc.vector.memset(kt_sb[:, :, :, S:], 0.0)
            for w_sb, dst, pad in ((wq_sb, qt_sb, False), (wk_sb, kt_sb, True)):
                for m in range(MT):
                    ps = pp_qk.tile([128, 2, S], F32, tag="proj", name=f"psqk_{pr}_{m}")
                    for k in range(KT):
                        nc.tensor.matmul(
                            out=ps,
                            lhsT=w_sb[:, k, m * 128 : (m + 1) * 128],
                            rhs=xt[:, k],
                            start=(k == 0),
                            stop=(k == KT - 1),
                        )
                    if pad:
                        nc.vector.tensor_copy(out=dst[:, m, :, 0:S], in_=ps)
                    else:
                        nc.vector.tensor_copy(out=dst[:, m], in_=ps)
                    yield

            # V: natural [s, e] layout with a ones column appended per head
            vpads_by_b = []
            for bi in range(2):
                vpads = []
                for st, (ss, sn) in enumerate(S_TILES):
                    vp = v_pool.tile(
                        [128, H, D + 1], BF16, tag=f"vpad{st}", name=f"vp{st}_{pr}_{bi}"
                    )
                    nc.vector.memset(vp[:sn, :, D : D + 1], 1.0)
                    for n in range(2):
                        ps = pp_v.tile([128, 512], F32, tag="proj", name=f"psv_{pr}_{bi}_{st}_{n}")
                        for k in range(KT):
                            nc.tensor.matmul(
                                out=ps[:sn],
                                lhsT=xt[:, k, bi, ss : ss + sn],
                                rhs=wv_sb[:, k, n * 512 : (n + 1) * 512],
                                start=(k == 0),
                                stop=(k == KT - 1),
                            )
                        nc.vector.tensor_copy(
                            out=vp[:sn, n * 8 : (n + 1) * 8, 0:D],
                            in_=ps[:sn].rearrange("p (h d) -> p h d", d=D),
                        )
                        yield
                    vpads.append(vp)
                vpads_by_b.append(vpads)
            stage[pr] = (qt_sb, kt_sb, vpads_by_b)

        def attn_gen(pr):
            """Attention + LayerNorm for both batches of pair `pr`; yields per
            head so pair pr+1 projection matmuls can fill PE idle gaps."""
            qt_sb, kt_sb, vpads_by_b = stage.pop(pr)
            o_by_b = []
            for bi in range(2):
                b = 2 * pr + bi
                vpads = vpads_by_b[bi]
                o_tiles = [
                    o_pool.tile([128, E], F32, tag=f"o{st}", name=f"o{st}_{b}")
                    for st, _ in enumerate(S_TILES)
                ]
                o_by_b.append(o_tiles)
                ps_o4 = None
                for h in range(H):
                    m, r0 = h // 2, (h % 2) * D
                    # scores.T[j, i], both j-tiles in one psum bank -> one
                    # w-mul and one exp per head. The jt=1 stationary is the
                    # zero-padded 128-wide kt slice, so all psum rows are
                    # written (zeros beyond row 51).
                    ps_s = pp_s.tile([128, 2, S], F32, tag="s", name=f"pss_{b}_{h}")
                    for jt in range(2):
                        nc.tensor.matmul(
                            out=ps_s[:, jt],
                            lhsT=kt_sb[r0 : r0 + D, m, bi, jt * 128 : (jt + 1) * 128],
                            rhs=qt_sb[r0 : r0 + D, m, bi, :],
                            start=True,
                            stop=True,
                        )
                    nc.vector.tensor_mul(out=ps_s, in0=ps_s, in1=wsc_sb)
                    p_t = p_pool.tile([128, 2, S], BF16, tag="p", name=f"p_{b}_{h}")
                    nc.scalar.activation(out=p_t, in_=ps_s, func=AF.Exp)

                    # PV: 4 heads share a psum bank: [i, 4, 65] where col 64 of
                    # each head is the softmax denominator (ones column in V).
                    hc = h % 4
                    if hc == 0:
                        ps_o4 = [
                            pp_o.tile([128, 4, D + 1], F32, tag="po", name=f"pso_{b}_{h}_{it}")
                            for it, _ in enumerate(S_TILES)
                        ]
                    for it, (is_, in_n) in enumerate(S_TILES):
                        for jt, (js, jn) in enumerate(S_TILES):
                            nc.tensor.matmul(
                                out=ps_o4[it][:in_n, hc],
                                lhsT=p_t[:jn, jt, is_ : is_ + in_n],
                                rhs=vpads[jt][:jn, h],
                                start=(jt == 0),
                                stop=(jt == 1),
                            )
                    if hc == 3:
                        # Batched normalize for the 4-head group: one
                        # reciprocal of the 4 denominators, one broadcast
                        # multiply writing [i, 4*64] of the output tile.
                        g0 = (h - 3) * D
                        for it, (is_, in_n) in enumerate(S_TILES):
                            rec = r_pool.tile([128, 4], F32, tag="rec4", name=f"rc_{b}_{h}_{it}")
                            nc.vector.reciprocal(
                                out=rec[:in_n], in_=ps_o4[it][:in_n, :, D]
                            )
                            rb = rec[:in_n]
                            rbc = bass.AP(
                                tensor=rb.tensor,
                                offset=rb.offset,
                                ap=list(rb.ap) + [[0, D]],
                            )
                            nc.vector.tensor_mul(
                                out=o_tiles[it][:in_n, g0 : g0 + 4 * D].rearrange(
                                    "p (h d) -> p h d", d=D
                                ),
                                in0=ps_o4[it][:in_n, :, 0:D],
                                in1=rbc,
                            )
                    yield "h"

            # LayerNorm for both batches last: keeps the ACT table warm (all
            # Exp during attention, then all Sqrt).
            for bi in range(2):
                b = 2 * pr + bi
                for it, (is_, in_n) in enumerate(S_TILES):
                    o_sb = o_by_b[bi][it]
                    stats = ln_pool.tile([128, 2, 6], F32, tag="stats", name=f"st_{b}_{it}")
                    mv = ln_pool.tile([128, 2], F32, tag="mv", name=f"mv_{b}_{it}")
                    nc.vector.bn_stats(out=stats[:in_n, 0], in_=o_sb[:in_n, 0:512])
                    nc.vector.bn_stats(out=stats[:in_n, 1], in_=o_sb[:in_n, 512:E])
                    nc.vector.bn_aggr(out=mv[:in_n], in_=stats[:in_n])
                    rstd = r_pool.tile([128, 1], F32, tag="rstd", name=f"rs_{b}_{it}")
                    nc.scalar.activation(
                        out=rstd[:in_n], in_=mv[:in_n, 1:2], func=AF.Sqrt, bias=eps_t[:in_n]
                    )
                    rrstd = r_pool.tile([128, 1], F32, tag="rrstd", name=f"rr_{b}_{it}")
                    nc.vector.reciprocal(out=rrstd[:in_n], in_=rstd[:in_n])
                    nc.vector.tensor_scalar(
                        out=o_sb[:in_n],
                        in0=o_sb[:in_n],
                        scalar1=mv[:in_n, 0:1],
                        scalar2=rrstd[:in_n],
                        op0=ALU.subtract,
                        op1=ALU.mult,
                    )
                    if apply_gb:
                        nc.vector.tensor_mul(out=o_sb[:in_n], in0=o_sb[:in_n], in1=gamma_b[:in_n])
                        nc.vector.tensor_add(out=o_sb[:in_n], in0=o_sb[:in_n], in1=beta_b[:in_n])
                    nc.sync.dma_start(out=out[b, is_ : is_ + in_n], in_=o_sb[:in_n])
                yield "ln"

        # Software pipeline: attention(p) interleaved with projections of later
        # pairs (depth 2) so the PE instruction stream never idles long enough
        # to re-throttle the HAM clock gate — including at pair boundaries.
        from collections import deque

        N_ATTN_CHUNKS = 2 * H + 2    # 34 yields per attn_gen
        for _ in proj_gen(0):
            pass
        pending: deque = deque()
        next_pair = 1

        def push_next():
            nonlocal next_pair
            if next_pair < npair:
                pending.append(proj_gen(next_pair))
                next_pair += 1

        def advance_one():
            while pending:
                if next(pending[0], "END") == "END":
                    pending.popleft()
                    push_next()
                    continue
                return True
            return False

        push_next()
        for p in range(npair):
            ag = attn_gen(p)
            acc = 0
            for tok in ag:
                # Distribute ~24 proj chunks per pair: 18 across the 32 head
                # yields, 3 at each LN yield (where attention gives the PE the
                # least work).
                if tok == "ln":
                    for _ in range(3):
                        if not advance_one():
                            break
                else:
                    acc += 18
                    while acc >= 32:
                        if not advance_one():
                            break
                        acc -= 32
            # Boundary: proj(p+1) must be fully emitted before attention(p+1).
            while (p + 1) < npair and (p + 1) not in stage:
                if not advance_one():
                    break


_NC_CACHE: dict = {}


def _get_nc(bpc: int = BPC, apply_gb: bool = True) -> bass.Bass:
    key = (bpc, apply_gb)
    if key not in _NC_CACHE:
        _NC_CACHE[key] = _build_kernel(bpc, apply_gb)
    return _NC_CACHE[key]


def _host_inputs(x, Wq, Wk, Wv, gamma, beta):
    import ml_dtypes

    bf16 = ml_dtypes.bfloat16
    x = np.asarray(x, dtype=np.float32)
    xT = np.ascontiguousarray(x.transpose(0, 2, 1)).astype(bf16)  # [B, E, S]
    idx = np.arange(S, dtype=np.float32)
    wsc = (np.abs(idx[None, :] - idx[:, None]) / S * SCALE).astype(np.float32)
    common = {
        "wqT": np.ascontiguousarray(np.asarray(Wq, np.float32).T).astype(bf16),
        "wkT": np.ascontiguousarray(np.asarray(Wk, np.float32).T).astype(bf16),
        "wvT": np.ascontiguousarray(np.asarray(Wv, np.float32).T).astype(bf16),
        "wsc": wsc,
        "gamma": np.asarray(gamma, np.float32),
        "beta": np.asarray(beta, np.float32),
    }
    return xT, common


def run(inputs: dict, trace: bool = False, trace_dir: str | None = None):
    """Run the SPMD kernel on 8 cores. Returns (full_output, exec_time_ns)."""
    xT, common = _host_inputs(**inputs)
    in_maps = [
        {**common, "xT": np.ascontiguousarray(xT[c * BPC : (c + 1) * BPC])}
        for c in range(NCORES)
    ]
    apply_gb = not (
        np.all(np.asarray(inputs["gamma"]) == 1.0)
        and np.all(np.asarray(inputs["beta"]) == 0.0)
    )
    nc = _get_nc(BPC, apply_gb)
    res = run_bass_kernel_spmd(
        nc, in_maps, core_ids=list(range(NCORES)), trace=trace, tmpdir=trace_dir
    )
    full = np.concatenate([res.results[c]["out"] for c in range(NCORES)], axis=0)
    return full.astype(np.float32), res.exec_time_ns


def kernel(x, Wq, Wk, Wv, gamma, beta):
    full, _ = run(dict(x=x, Wq=Wq, Wk=Wk, Wv=Wv, gamma=gamma, beta=beta))
    return full



# revision 10
# speedup vs baseline: 1.0201x; 1.0201x over previous
"""Trainium2 Bass kernel for nn_Attention_6468220748045.

Computes, per batch item: QKV projection -> per-head scaled attention with a
multiplicative positional bias w[i,j] = |i-j|/S -> softmax -> attn @ V ->
LayerNorm over the embedding dim.

Sharding: pure data-parallel over batch. B=128 splits as 16 batch items per
core across 8 NeuronCores; no collectives needed.

Per-core kernel layout (v2):
  - Q.T/K.T projections run in fp8(e4m3) with perf_mode=DoubleRow: the
    contraction packs 2 rows per PE cell, so E=1024 takes 4 matmuls of 256
    (layout [128, kt, 2, ...], k = 256*kt + 2p + o) instead of 8 of 128 —
    half the row time AND half the LDWEIGHTS. Wq/Wk are pre-scaled by 32 on
    the host (keeps fp8 out of the subnormal range); the 2^-10 descale is
    folded into the host-precomputed positional-bias table. Accuracy was
    validated against an fp64 model: rel err ~1e-2 vs the 2e-2 budget.
  - V stays bf16 (fp8 V fails the error budget): stationary = x.T tile,
    moving = Wv.T ([e_in,128 x 512]) giving V in natural [s, e] orientation.
  - V projection is emitted INSIDE the attention window of its own pair
    (interleaved with scores), so the final pair's attention keeps the PE
    dense — previously the tail ran at half clock (HAM throttle) because it
    had no projection work left to interleave.
  - Scores: s.T[j, i] = kt.T @ qt per head in bf16, multiplied by the
    host-precomputed scale*w[j, i] (VectorE), exponentiated on ScalarE.
  - Softmax denominator comes free from the PV matmul: V is stored padded
    [s, H, 65] with a ones column, so out[:, 64] = sum_j p[j, i].
  - psum->SBUF casts for qt/kt and V run on ScalarE (ACT Copy), keeping
    VectorE (wsc-mul, normalize, LN) off the critical path.
"""

import numpy as np

import concourse.bass as bass
import concourse.tile as tile
from concourse import bacc, mybir
from concourse.bass_utils import run_bass_kernel_spmd

# Problem constants (hardcoded per the self-contained-kernel contract).
B, S, E, H, D = 128, 179, 1024, 16, 64
NCORES = 8
BPC = B // NCORES          # batches per core = 16
NPAIR = BPC // 2           # batch pairs per core = 8
KT8 = 4                    # fp8 DoubleRow contraction tiles (256 each)
KT = E // 128              # bf16 contraction tiles over e_in = 8
MT = E // 128              # output tiles over e_out = 8
SP = 256                   # kt s-padded width
S0 = 128                   # first s-tile size
S1 = S - S0                # second s-tile size = 51
S_TILES = ((0, S0), (S0, S1))
LN_EPS = 1e-5
SCALE = float(E) ** -0.5
W8_SCALE = 32.0            # host pre-scale on Wq/Wk before fp8 quantization

F32 = mybir.dt.float32
BF16 = mybir.dt.bfloat16
FP8 = mybir.dt.float8e4

AF = mybir.ActivationFunctionType
ALU = mybir.AluOpType
DR = mybir.MatmulPerfMode.DoubleRow


def _build_kernel(bpc: int = BPC, apply_gb: bool = True) -> bass.Bass:
    npair = bpc // 2
    nc = bacc.Bacc()

    x8 = nc.dram_tensor("x8", [KT8, 128, npair, 2, 2, S], FP8, kind="ExternalInput").ap()
    xb = nc.dram_tensor("xb", [bpc, E, S], BF16, kind="ExternalInput").ap()
    wq8 = nc.dram_tensor("wq8", [E, E], FP8, kind="ExternalInput").ap()
    wk8 = nc.dram_tensor("wk8", [E, E], FP8, kind="ExternalInput").ap()
    wvT = nc.dram_tensor("wvT", [E, E], BF16, kind="ExternalInput").ap()
    wsc = nc.dram_tensor("wsc", [S, S], F32, kind="ExternalInput").ap()
    gamma = nc.dram_tensor("gamma", [E], F32, kind="ExternalInput").ap()
    beta = nc.dram_tensor("beta", [E], F32, kind="ExternalInput").ap()
    out = nc.dram_tensor("out", [bpc, S, E], F32, kind="ExternalOutput").ap()

    with tile.TileContext(nc) as tc:
        _emit(tc, npair, out, x8, xb, wq8, wk8, wvT, wsc, gamma, beta, apply_gb)
    nc.compile()
    return nc


def _emit(tc, npair, out, x8, xb, wq8, wk8, wvT, wsc, gamma, beta, apply_gb):
    nc = tc.nc
    from collections import deque
    from contextlib import ExitStack

    with ExitStack() as ctx:
        singles = ctx.enter_context(tc.tile_pool(name="singles", bufs=1))
        x8_pool = ctx.enter_context(tc.tile_pool(name="x8", bufs=3))
        xb_pool = ctx.enter_context(tc.tile_pool(name="xb", bufs=3))
        qk_pool = ctx.enter_context(tc.tile_pool(name="qk", bufs=2))
        v_pool = ctx.enter_context(tc.tile_pool(name="v", bufs=4))
        p_pool = ctx.enter_context(tc.tile_pool(name="p", bufs=6))
        o_pool = ctx.enter_context(tc.tile_pool(name="o", bufs=3))
        ln_pool = ctx.enter_context(tc.tile_pool(name="ln", bufs=4))
        r_pool = ctx.enter_context(tc.tile_pool(name="r", bufs=8))

        # QK and V projection psums share one pool/tag (their emission
        # interleaves), freeing a PSUM bank so the scores pool gets 3 buffers.
        pp_proj = ctx.enter_context(tc.tile_pool(name="pp_proj", bufs=3, space="PSUM"))
        pp_s = ctx.enter_context(tc.tile_pool(name="pp_s", bufs=3, space="PSUM"))
        pp_o = ctx.enter_context(tc.tile_pool(name="pp_o", bufs=2, space="PSUM"))

        # --- resident tensors -------------------------------------------------
        # x8 is pre-laid-out on host as [k, p, pair, o, bi, s] (DoubleRow
        # e_in index = 256*k + 2*p + o) so each pair's DMA reads one
        # contiguous 716B run per partition.
        xbsrc = xb.rearrange("b (k p) s -> k p b s", p=128)
        w8q_sb = singles.tile([128, KT8, 2, E], FP8, tag="w8q")
        w8k_sb = singles.tile([128, KT8, 2, E], FP8, tag="w8k")
        wv_sb = singles.tile([128, KT, E], BF16, tag="wv")

        # DMA order matters for startup latency: wq8 + pair-0 x8 first (the
        # first Q.T matmuls need only these), then wk8, then V-path tensors.
        wq8_src = wq8.rearrange("(k p o) e -> k p o e", p=128, o=2)
        wk8_src = wk8.rearrange("(k p o) e -> k p o e", p=128, o=2)
        for k in range(KT8):
            nc.sync.dma_start(out=w8q_sb[:, k], in_=wq8_src[k])
        x8t0 = x8_pool.tile([128, KT8, 2, 2, S], FP8, tag="x8t", name="x8t_0")
        for k in range(KT8):
            nc.sync.dma_start(out=x8t0[:, k], in_=x8[k, :, 0])
        for k in range(KT8):
            nc.sync.dma_start(out=w8k_sb[:, k], in_=wk8_src[k])
        xbt0 = xb_pool.tile([128, KT, 2, S], BF16, tag="xbt", name="xbt_0")
        for k in range(KT):
            nc.sync.dma_start(out=xbt0[:, k], in_=xbsrc[k, :, 0:2, :])
        wv_src = wvT.rearrange("(k p) e -> k p e", p=128)
        for k in range(KT):
            nc.sync.dma_start(out=wv_sb[:, k], in_=wv_src[k])

        # Positional bias (includes softmax scale and fp8 descale):
        # [j partition, jt, i]
        wsc_sb = singles.tile([128, 2, S], F32, tag="wsc")
        nc.vector.memset(wsc_sb[:, 1], 0.0)
        nc.sync.dma_start(out=wsc_sb[:, 0], in_=wsc[0:S0])
        nc.sync.dma_start(out=wsc_sb[0:S1, 1], in_=wsc[S0:S])

        if apply_gb:
            gamma_b = singles.tile([128, E], F32, tag="gamma")
            beta_b = singles.tile([128, E], F32, tag="beta")
            nc.sync.dma_start(
                out=gamma_b,
                in_=bass.AP(tensor=gamma.tensor, offset=gamma.offset, ap=[[0, 128]] + gamma.ap),
            )
            nc.sync.dma_start(
                out=beta_b,
                in_=bass.AP(tensor=beta.tensor, offset=beta.offset, ap=[[0, 128]] + beta.ap),
            )
        eps_t = singles.tile([128, 1], F32, tag="eps")
        nc.vector.memset(eps_t, LN_EPS)

        # Per-pair SBUF products handed from the QK-projection stage to the
        # attention stage (software pipeline).
        stage: dict = {}

        def proj_gen(pr):
            """fp8 DoubleRow Q.T/K.T projections for batch pair `pr`; yields
            after each psum group so attention of pair pr-1 interleaves."""
            if pr == 0:
                x8t, xbt = x8t0, xbt0
            else:
                x8t = x8_pool.tile([128, KT8, 2, 2, S], FP8, tag="x8t", name=f"x8t_{pr}")
                for k in range(KT8):
                    nc.sync.dma_start(out=x8t[:, k], in_=x8[k, :, pr])
                xbt = xb_pool.tile([128, KT, 2, S], BF16, tag="xbt", name=f"xbt_{pr}")
                for k in range(KT):
                    nc.sync.dma_start(
                        out=xbt[:, k], in_=xbsrc[k, :, 2 * pr : 2 * pr + 2, :]
                    )

            # kt is stored s-padded to 2*128 per batch with a zeroed tail, so
            # the jt=1 scores matmul has a full 128-wide stationary and writes
            # all 128 psum rows (zeros beyond row 51) — no per-head memset.
            qt_sb = qk_pool.tile([128, MT, 2, S], BF16, tag="qt", name=f"qt_{pr}")
            kt_sb = qk_pool.tile([128, MT, 2, SP], BF16, tag="kt", name=f"kt_{pr}")
            nc.vector.memset(kt_sb[:, :, :, S:], 0.0)
            for w8_sb, dst, pad in ((w8q_sb, qt_sb, False), (w8k_sb, kt_sb, True)):
                for m in range(MT):
                    ps = pp_proj.tile([128, 2, S], F32, tag="proj", name=f"psqk_{pr}_{m}")
                    for k in range(KT8):
                        nc.tensor.matmul(
                            out=ps,
                            lhsT=w8_sb[:, k, :, m * 128 : (m + 1) * 128],
                            rhs=x8t[:, k],
                            start=(k == 0),
                            stop=(k == KT8 - 1),
                            perf_mode=DR,
                        )
                    if pad:
                        nc.scalar.copy(out=dst[:, m, :, 0:S], in_=ps)
                    else:
                        nc.scalar.copy(out=dst[:, m], in_=ps)
                    yield
            stage[pr] = (qt_sb, kt_sb, xbt)

        def attn_gen(pr):
            """V projection + attention + LayerNorm for both batches of pair
            `pr`; yields per unit so pair pr+1 QK projections fill PE gaps."""
            qt_sb, kt_sb, xbt = stage.pop(pr)

            # V tiles: natural [s, e] layout, ones column appended per head.
            vps = [
                [
                    v_pool.tile([128, H, D + 1], BF16, tag=f"vp{st}", name=f"vp{st}_{pr}_{bi}")
                    for st in (0, 1)
                ]
                for bi in (0, 1)
            ]
            for bi in (0, 1):
                for st, (ss, sn) in enumerate(S_TILES):
                    nc.vector.memset(vps[bi][st][:sn, :, D : D + 1], 1.0)

            def v_full(bi, n):
                ps = pp_proj.tile([128, 512], F32, tag="proj", name=f"psv_{pr}_{bi}_{n}")
                for k in range(KT):
                    nc.tensor.matmul(
                        out=ps,
                        lhsT=xbt[:, k, bi, 0:S0],
                        rhs=wv_sb[:, k, n * 512 : (n + 1) * 512],
                        start=(k == 0),
                        stop=(k == KT - 1),
                    )
                nc.scalar.copy(
                    out=vps[bi][0][:, n * 8 : (n + 1) * 8, 0:D],
                    in_=ps.rearrange("p (h d) -> p h d", d=D),
                )

            def v_tail(bi, n):
                ps = pp_proj.tile([128, 512], F32, tag="proj", name=f"psvt_{pr}_{bi}_{n}")
                for k in range(KT):
                    nc.tensor.matmul(
                        out=ps[:S1],
                        lhsT=xbt[:, k, bi, S0:S],
                        rhs=wv_sb[:, k, n * 512 : (n + 1) * 512],
                        start=(k == 0),
                        stop=(k == KT - 1),
                    )
                nc.scalar.copy(
                    out=vps[bi][1][:S1, n * 8 : (n + 1) * 8, 0:D],
                    in_=ps[:S1].rearrange("p (h d) -> p h d", d=D),
                )

            # b0's V complete after 4 groups, b1's after 8.
            v_thunks = deque(
                [
                    lambda bi=bi, n=n, tl=tl: (v_tail if tl else v_full)(bi, n)
                    for bi in (0, 1)
                    for tl in (0, 1)
                    for n in (0, 1)
                ]
            )
            v_emitted = 0
            v_need = {0: 4, 1: 8}

            o_tiles = {
                bi: [
                    o_pool.tile([128, E], F32, tag=f"o{st}", name=f"o{st}_{2*pr+bi}")
                    for st, _ in enumerate(S_TILES)
                ]
                for bi in (0, 1)
            }
            ps_o4 = {}

            def scores_exp(bi, h):
                m, r0 = h // 2, (h % 2) * D
                ps_s = pp_s.tile([128, 2, S], F32, tag="s", name=f"pss_{2*pr+bi}_{h}")
                for jt in range(2):
                    nc.tensor.matmul(
                        out=ps_s[:, jt],
                        lhsT=kt_sb[r0 : r0 + D, m, bi, jt * 128 : (jt + 1) * 128],
                        rhs=qt_sb[r0 : r0 + D, m, bi, :],
                        start=True,
                        stop=True,
                    )
                nc.vector.tensor_mul(out=ps_s, in0=ps_s, in1=wsc_sb)
                p_t = p_pool.tile([128, 2, S], BF16, tag="p", name=f"p_{2*pr+bi}_{h}")
                nc.scalar.activation(out=p_t, in_=ps_s, func=AF.Exp)
                return p_t

            def pv(bi, h, p_t):
                b = 2 * pr + bi
                hc = h % 4
                if hc == 0:
                    ps_o4[bi] = [
                        pp_o.tile([128, 4, D + 1], F32, tag="po", name=f"pso_{b}_{h}_{it}")
                        for it, _ in enumerate(S_TILES)
                    ]
                for it, (is_, in_n) in enumerate(S_TILES):
                    for jt, (js, jn) in enumerate(S_TILES):
                        nc.tensor.matmul(
                            out=ps_o4[bi][it][:in_n, hc],
                            lhsT=p_t[:jn, jt, is_ : is_ + in_n],
                            rhs=vps[bi][jt][:jn, h],
                            start=(jt == 0),
                            stop=(jt == 1),
                        )
                if hc == 3:
                    # Batched normalize for the 4-head group: one reciprocal
                    # of the 4 denominators, one broadcast multiply.
                    g0 = (h - 3) * D
                    for it, (is_, in_n) in enumerate(S_TILES):
                        rec = r_pool.tile([128, 4], F32, tag="rec4", name=f"rc_{b}_{h}_{it}")
                        nc.vector.reciprocal(out=rec[:in_n], in_=ps_o4[bi][it][:in_n, :, D])
                        rb = rec[:in_n]
                        rbc = bass.AP(
                            tensor=rb.tensor,
                            offset=rb.offset,
                            ap=list(rb.ap) + [[0, D]],
                        )
                        nc.vector.tensor_mul(
                            out=o_tiles[bi][it][:in_n, g0 : g0 + 4 * D].rearrange(
                                "p (h d) -> p h d", d=D
                            ),
                            in0=ps_o4[bi][it][:in_n, :, 0:D],
                            in1=rbc,
                        )

            def ln_unit(bi, it):
                b = 2 * pr + bi
                is_, in_n = S_TILES[it]
                o_sb = o_tiles[bi][it]
                stats = ln_pool.tile([128, 2, 6], F32, tag="stats", name=f"st_{b}_{it}")
                mv = ln_pool.tile([128, 2], F32, tag="mv", name=f"mv_{b}_{it}")
                nc.vector.bn_stats(out=stats[:in_n, 0], in_=o_sb[:in_n, 0:512])
                nc.vector.bn_stats(out=stats[:in_n, 1], in_=o_sb[:in_n, 512:E])
                nc.vector.bn_aggr(out=mv[:in_n], in_=stats[:in_n])
                rstd = r_pool.tile([128, 1], F32, tag="rstd", name=f"rs_{b}_{it}")
                nc.scalar.activation(
                    out=rstd[:in_n], in_=mv[:in_n, 1:2], func=AF.Sqrt, bias=eps_t[:in_n]
                )
                rrstd = r_pool.tile([128, 1], F32, tag="rrstd", name=f"rr_{b}_{it}")
                nc.vector.reciprocal(out=rrstd[:in_n], in_=rstd[:in_n])
                nc.vector.tensor_scalar(
                    out=o_sb[:in_n],
                    in0=o_sb[:in_n],
                    scalar1=mv[:in_n, 0:1],
                    scalar2=rrstd[:in_n],
                    op0=ALU.subtract,
                    op1=ALU.mult,
                )
                if apply_gb:
                    nc.vector.tensor_mul(out=o_sb[:in_n], in0=o_sb[:in_n], in1=gamma_b[:in_n])
                    nc.vector.tensor_add(out=o_sb[:in_n], in0=o_sb[:in_n], in1=beta_b[:in_n])
                nc.sync.dma_start(out=out[b, is_ : is_ + in_n], in_=o_sb[:in_n])

            # Head loop. V groups drain during b0's early heads; PV for a
            # batch starts once its V is complete; LN(b0) lands inside b1's
            # head loop so the vector tail stays short.
            pv_queue = deque()
            for bi in (0, 1):
                for h in range(H):
                    pv_queue.append((bi, h, scores_exp(bi, h)))
                    yield "s"
                    if v_thunks:
                        v_thunks.popleft()()
                        v_emitted += 1
                        yield "v"
                    while pv_queue and v_emitted >= v_need[pv_queue[0][0]]:
                        qbi, qh, qp = pv_queue.popleft()
                        pv(qbi, qh, qp)
                        yield "pv"
                    if bi == 1 and h == 4:
                        ln_unit(0, 0)
                        yield "ln"
                    if bi == 1 and h == 8:
                        ln_unit(0, 1)
                        yield "ln"
            for it in (0, 1):
                ln_unit(1, it)
                yield "ln"

        # Software pipeline driver: QK projections of pair p+1 interleave
        # into the attention window of pair p. Lookahead is strictly depth-1 —
        # emitting proj(p+2) inside attention(p) would recycle qt/kt buffers
        # that attention(p) still reads (emission-order deadlock).
        for _ in proj_gen(0):
            pass
        progen = proj_gen(1) if npair > 1 else None
        for p in range(npair):
            yi = 0
            for tok in attn_gen(p):
                yi += 1
                if progen is not None and (tok == "ln" or yi % 5 == 0):
                    if next(progen, "END") == "END":
                        progen = None
            # Boundary: proj(p+1) must be fully emitted before attention(p+1).
            while progen is not None:
                if next(progen, "END") == "END":
                    progen = None
            progen = proj_gen(p + 2) if (p + 2) < npair else None


_NC_CACHE: dict = {}


def _get_nc(bpc: int = BPC, apply_gb: bool = True) -> bass.Bass:
    key = (bpc, apply_gb)
    if key not in _NC_CACHE:
        _NC_CACHE[key] = _build_kernel(bpc, apply_gb)
    return _NC_CACHE[key]


def _host_inputs(x, Wq, Wk, Wv, gamma, beta):
    import ml_dtypes

    bf16 = ml_dtypes.bfloat16
    f8 = ml_dtypes.float8_e4m3fn
    x = np.asarray(x, dtype=np.float32)
    xT = np.ascontiguousarray(x.transpose(0, 2, 1))  # [B, E, S] f32
    # fp8 copy in DoubleRow layout [core, k, p, pair, o, bi, s] where
    # e_in = 256*k + 2*p + o and b = (core*NPAIR + pair)*2 + bi.
    x8 = (
        xT.astype(f8)
        .reshape(NCORES, NPAIR, 2, KT8, 128, 2, S)
        .transpose(0, 3, 4, 1, 5, 2, 6)
    )
    idx = np.arange(S, dtype=np.float32)
    wsc = (
        np.abs(idx[None, :] - idx[:, None]) / S * SCALE / (W8_SCALE * W8_SCALE)
    ).astype(np.float32)
    common = {
        "wq8": np.ascontiguousarray(np.asarray(Wq, np.float32).T * W8_SCALE).astype(f8),
        "wk8": np.ascontiguousarray(np.asarray(Wk, np.float32).T * W8_SCALE).astype(f8),
        "wvT": np.ascontiguousarray(np.asarray(Wv, np.float32).T).astype(bf16),
        "wsc": wsc,
        "gamma": np.asarray(gamma, np.float32),
        "beta": np.asarray(beta, np.float32),
    }
    return x8, xT.astype(bf16), common


def run(inputs: dict, trace: bool = False, trace_dir: str | None = None):
    """Run the SPMD kernel on 8 cores. Returns (full_output, exec_time_ns)."""
    x8, xb, common = _host_inputs(**inputs)
    in_maps = [
        {
            **common,
            "x8": np.ascontiguousarray(x8[c]),
            "xb": np.ascontiguousarray(xb[c * BPC : (c + 1) * BPC]),
        }
        for c in range(NCORES)
    ]
    apply_gb = not (
        np.all(np.asarray(inputs["gamma"]) == 1.0)
        and np.all(np.asarray(inputs["beta"]) == 0.0)
    )
    nc = _get_nc(BPC, apply_gb)
    res = run_bass_kernel_spmd(
        nc, in_maps, core_ids=list(range(NCORES)), trace=trace, tmpdir=trace_dir
    )
    full = np.concatenate([res.results[c]["out"] for c in range(NCORES)], axis=0)
    return full.astype(np.float32), res.exec_time_ns


def kernel(x, Wq, Wk, Wv, gamma, beta):
    full, _ = run(dict(x=x, Wq=Wq, Wk=Wk, Wv=Wv, gamma=gamma, beta=beta))
    return full
